# revision 15
# baseline (speedup 1.0000x reference)
"""DynamicCenterLoss on Trainium2 (Bass/Tile), 8-core SPMD.

Strategy: `batch` is sorted, so shard at batch boundaries -> core b owns
batch b. Host uploads two fp8 tensors per core: feat [Npad, 64] and
onehot(target) [Npad, 13] (4x less HBM than f32 feat; no on-device cast
or broadcast-is_equal).  Device computes only the O(N) reductions:

  - fsum[13,64] per-class feature sums: per-128-point chunk matmul
    acc += onehot^T @ feat, spread over 4 PE column groups
    (tile_position) so 4 chunk-matmuls stream concurrently.
  - S = sum ||f||^2: each DMA tile is squared+reduced by ONE engine --
    ScalarE (activation Square + accum_out) or the DVE (custom-DVE
    affine_mul_reduce in0*in1 + accum; the native tensor_tensor_reduce
    ISA op crashes this silicon) -- with tile ownership balanced by the
    engines' 1.2 vs 0.96 GHz rates.  Flat step-1 access patterns.

Counts/presence (ccnt, cnt_b) are exact host-side bincounts of
target/batch; the tiny (13,64) pairwise-center tail runs on the host in
f64 as part of the gather.  fp8e4 quantization of feat gives rel err
~3.5e-4 on the loss (tolerance 2e-2).
"""

import numpy as np
import ml_dtypes

import concourse.bass as bass
import concourse.bacc as bacc
import concourse.tile as tile
from concourse import mybir
from concourse.bass_utils import run_bass_kernel_spmd

P = 128
D = 64
C = 13
B = 8
N_CORES = 8
MARGIN = 0.5
INTRA_W = 1.0
INTER_W = 1.0
LOSS_W = 0.01
IGNORE = -1
NGRP = 4  # PE column groups

f32 = mybir.dt.float32
f8 = mybir.dt.float8e4

F8NP = getattr(ml_dtypes, "float8_e4m3", ml_dtypes.float8_e4m3fn)


def _plan(T: int) -> list[tuple[int, int, str]]:
    """(t0, tt, owner) tiles. Small first tile (compute starts early),
    small last tile (less square work exposed after the DMA stream);
    owners greedy-balanced for ScalarE 1.2 GHz vs DVE 0.96 GHz."""
    szs = []
    rem = T
    for s in (32, 96):
        t = min(s, rem)
        if t:
            szs.append(t)
            rem -= t
    while rem > 160:
        szs.append(128)
        rem -= 128
    if rem > 32:
        szs.append(rem - 24)
        rem = 24
    if rem:
        szs.append(rem)

    out = []
    t0 = 0
    # tile 0 -> DVE: its square runs while ScalarE loads the Square table
    ts = 1280.0
    tv = 0.0
    for i, sz in enumerate(szs):
        cs = ts + sz * D / 1.2 + 830  # init + read-accumulator
        cv = tv + sz * D * 1.04 / 0.96 + 350
        if i > 0 and cs <= cv:
            out.append((t0, sz, "S"))
            ts = cs
        else:
            out.append((t0, sz, "V"))
            tv = cv
        t0 += sz
    return out


def build_nc(T: int) -> bass.Bass:
    plan = _plan(T)
    ntiles = len(plan)
    K = ntiles  # one accumulator column per tile
    TTMAX = max(tt for _, tt, _ in plan)
    Npad = P * T

    nc = bacc.Bacc("TRN2", target_bir_lowering=False)
    feat_h = nc.dram_tensor("feat8", [Npad, D], f8, kind="ExternalInput")
    oh_h = nc.dram_tensor("oh8", [Npad, C], f8, kind="ExternalInput")
    out_h = nc.dram_tensor("out", [P, D + K], f32, kind="ExternalOutput")

    # point n == (p, t), n = p*T + t -> per-partition contiguous, flat
    featv = feat_h[:, :].rearrange("(p t) d -> p (t d)", p=P)  # [128, T*64]
    ohv = oh_h[:, :].rearrange("(p t) d -> p (t d)", p=P)  # [128, T*13]

    first = [min(s for s in range(T) if s % NGRP == g) for g in range(NGRP)]
    last = [max(s for s in range(T) if s % NGRP == g) for g in range(NGRP)]

    with tile.TileContext(nc) as tc:
        with (
            tc.tile_pool(name="fin", bufs=1) as fp,
            tc.tile_pool(name="oh", bufs=1) as ohp,
            tc.tile_pool(name="io", bufs=6) as iop,
            tc.tile_pool(name="sq", bufs=4) as sqp,
            tc.tile_pool(name="acc", bufs=1, space="PSUM") as psa,
        ):
            final = fp.tile([P, D + K], f32)
            nc.vector.memset(final[:, :], 0.0)

            # all one-hots in one transfer on the scalar-engine HWDGE queue
            # so it drains in parallel with the feat stream on the sync queue
            oh_all = ohp.tile([P, T * C], f8)
            nc.scalar.dma_start(out=oh_all[:, :], in_=ohv[:, :])

            acc = psa.tile([32 * (NGRP - 1) + C, D], f32)  # [109, 64]

            step = 0
            for i, (t0, tt, owner) in enumerate(plan):
                io = iop.tile([P, TTMAX * D], f8, tag="io")
                nc.sync.dma_start(
                    out=io[:, : tt * D],
                    in_=featv[:, t0 * D : (t0 + tt) * D],
                )
                sq = sqp.tile([P, TTMAX * D], f8, tag="sq")
                if owner == "S":
                    nc.scalar.activation(
                        out=sq[:, : tt * D], in_=io[:, : tt * D],
                        func=mybir.ActivationFunctionType.Square,
                        accum_out=final[:, D + i : D + i + 1],
                    )
                else:
                    nc.vector.affine_mul_reduce(
                        out=sq[:, : tt * D],
                        accum_out=final[:, D + i : D + i + 1],
                        in0=io[:, : tt * D],
                        in1=io[:, : tt * D],
                        scale=1.0,
                        bias=0.0,
                    )
                for tl in range(tt):
                    g = step % NGRP
                    nc.tensor.matmul(
                        acc[32 * g : 32 * g + C, :],
                        lhsT=oh_all[:, step * C : (step + 1) * C],
                        rhs=io[:, tl * D : (tl + 1) * D],
                        start=(step == first[g]),
                        stop=(step == last[g]),
                        tile_position=(0, 32 * g),
                    )
                    step += 1

            nc.vector.tensor_copy(final[0 : 32 * 3 + C, 0:D], acc[:, :])
            nc.sync.dma_start(out=out_h[:, :], in_=final[:, :])
    nc.finalize()
    return nc


# set by test.py to capture profile info
TRACE = False
LAST = {}


def _ensure_ntff_hook():
    """The agent image's antenv lacks axon_hooks; synthesize it so
    run_bass_kernel_spmd(trace=True) can profile. Best-effort."""
    import sys
    import types

    try:
        from antenv.axon_hooks import get_axon_ntff_profile_hook  # noqa: F401
        return
    except ImportError:
        pass
    try:
        from trn_agent_boot.trn_boot import _ntff_profile_via_ctypes

        hook = _ntff_profile_via_ctypes("/opt/axon/libaxon_pjrt.so")
        mod = types.ModuleType("antenv.axon_hooks")
        mod._hook = hook
        mod.get_axon_ntff_profile_hook = lambda: mod._hook
        mod.set_axon_ntff_profile_hook = lambda h: setattr(mod, "_hook", h)
        sys.modules["antenv.axon_hooks"] = mod
        import antenv

        antenv.axon_hooks = mod
    except Exception as e:  # degrade: no profile, run still works
        print(f"ntff hook injection failed: {e}")


def kernel(pred=None, target=None, feat=None, batch=None, centers=None):
    target = np.asarray(target)
    feat = np.asarray(feat, dtype=np.float32)
    batch = np.asarray(batch)
    centers = np.asarray(centers, dtype=np.float64)

    # shard at batch boundaries: core b <- batch b (batch is sorted)
    bounds = np.searchsorted(batch, np.arange(B + 1))
    sizes = np.diff(bounds)
    T = int(max((int(sizes.max()) + P - 1) // P, 8))
    Npad = P * T

    valid_all = target != IGNORE
    feat8 = np.clip(feat, -240.0, 240.0).astype(F8NP)

    in_maps = []
    for b in range(B):
        lo, hi = int(bounds[b]), int(bounds[b + 1])
        n = hi - lo
        fb8 = np.zeros((Npad, D), dtype=F8NP)
        ohb = np.zeros((Npad, C), dtype=F8NP)
        v = valid_all[lo:hi]
        fb = feat8[lo:hi].copy()
        fb[~v] = 0
        fb8[:n] = fb
        tb = target[lo:hi]
        rows = np.nonzero(v)[0]
        ohb[rows, tb[rows]] = 1.0
        in_maps.append({"feat8": fb8, "oh8": ohb})

    nc = build_nc(T)
    if TRACE:
        _ensure_ntff_hook()
    res = run_bass_kernel_spmd(nc, in_maps, list(range(N_CORES)), trace=TRACE)
    LAST["results"] = res

    # host-side (exact, from int inputs only): per-batch/class counts
    seg = (batch.astype(np.int64) * C + np.where(valid_all, target, 0))[
        valid_all
    ]
    ccnt = np.bincount(seg, minlength=B * C).reshape(B, C).astype(np.float64)
    cnt_b = np.bincount(batch[valid_all], minlength=B).astype(np.float64)

    cen_sq = (centers**2).sum(axis=1)  # (13,)
    total_intra = 0.0
    total_inter = 0.0
    n_present = 0
    for b in range(B):
        o = np.asarray(res.results[b]["out"]).astype(np.float64)  # [128,64+K]
        fsum = np.zeros((C, D))
        for g in range(NGRP):
            fsum += o[32 * g : 32 * g + C, 0:D]
        S = o[:, D:].sum()
        if cnt_b[b] <= 0:
            continue
        n_present += 1
        tdot = (centers * fsum).sum()
        utot = (ccnt[b] * cen_sq).sum()
        total_intra += (S - 2.0 * tdot + utot) / max(cnt_b[b], 1.0)

        cm = fsum / np.maximum(ccnt[b], 1.0)[:, None]
        pres = ccnt[b] > 0
        dd2 = ((cm[:, None, :] - cm[None, :, :]) ** 2).sum(-1)
        pm = pres[:, None] & pres[None, :] & ~np.eye(C, dtype=bool)
        dist = np.sqrt(np.where(pm, dd2, 1.0))
        terms = np.where(pm, np.maximum(MARGIN - dist, 0.0), 0.0)
        npairs = pm.sum()
        total_inter += terms.sum() / max(npairs, 1)

    den = max(n_present, 1)
    loss = LOSS_W * (
        INTRA_W * total_intra / den + INTER_W * total_inter / den
    )
    return np.float32(loss)


# revision 20
# speedup vs baseline: 1.0900x; 1.0900x over previous
"""DynamicCenterLoss on Trainium2 (Bass/Tile), 8-core SPMD.

Strategy: `batch` is sorted, so shard at batch boundaries -> core b owns
batch b. Host uploads two fp8 tensors per core: feat [Npad, 64] and
onehot(target) [Npad, 13] (4x less HBM than f32 feat; no on-device cast
or broadcast-is_equal).  Device computes only the O(N) reductions:

  - fsum[13,64] per-class feature sums: per-128-point chunk matmul
    acc += onehot^T @ feat, spread over 4 PE column groups
    (tile_position) so 4 chunk-matmuls stream concurrently.
  - S = sum ||f||^2: each DMA tile is squared+reduced by ONE engine --
    ScalarE (activation Square + accum_out) or the DVE (custom-DVE
    affine_mul_reduce in0*in1 + accum; the native tensor_tensor_reduce
    ISA op crashes this silicon) -- with tile ownership balanced by the
    engines' 1.2 vs 0.96 GHz rates.  Flat step-1 access patterns.

Counts/presence (ccnt, cnt_b) are exact host-side bincounts of
target/batch; the tiny (13,64) pairwise-center tail runs on the host in
f64 as part of the gather.  fp8e4 quantization of feat gives rel err
~3.5e-4 on the loss (tolerance 2e-2).
"""

import numpy as np
import ml_dtypes

import concourse.bass as bass
import concourse.bacc as bacc
import concourse.tile as tile
from concourse import mybir
from concourse.bass_utils import run_bass_kernel_spmd

P = 128
D = 64
C = 13
B = 8
N_CORES = 8
MARGIN = 0.5
INTRA_W = 1.0
INTER_W = 1.0
LOSS_W = 0.01
IGNORE = -1
NGRP = 4  # PE column groups

f32 = mybir.dt.float32
f8 = mybir.dt.float8e4

F8NP = getattr(ml_dtypes, "float8_e4m3", ml_dtypes.float8_e4m3fn)


XS = 0.553  # fraction of each tile's steps squared on ScalarE (rest: DVE)


def _plan(T: int) -> list[tuple[int, int]]:
    """(t0, tt) tiles, all sizes even: small first tile (compute starts
    early), small last tile (less square work exposed after the stream)."""
    szs = []
    rem = T
    for s in (24, 96):
        t = min(s, rem)
        if t:
            szs.append(t)
            rem -= t
    while rem > 152:
        szs.append(128)
        rem -= 128
    if rem > 24:
        head = (rem - 24) & ~1
        if head:
            szs.append(head)
            rem -= head
    if rem:
        szs.append(rem)
    out = []
    t0 = 0
    for sz in szs:
        out.append((t0, sz))
        t0 += sz
    return out


def build_nc(T: int) -> bass.Bass:
    plan = _plan(T)
    ntiles = len(plan)
    K = 2 * ntiles  # accumulator columns (ScalarE + DVE per tile)
    TTMAX = max(tt for _, tt in plan)
    Npad = P * T

    nc = bacc.Bacc("TRN2", target_bir_lowering=False)
    feat_h = nc.dram_tensor("feat8", [Npad, D], f8, kind="ExternalInput")
    oh_h = nc.dram_tensor("oh8", [Npad, C], f8, kind="ExternalInput")
    out_h = nc.dram_tensor("out", [P, 2 * D + K], f32, kind="ExternalOutput")

    # point n == (p, t), n = p*T + t -> per-partition contiguous, flat
    featv = feat_h[:, :].rearrange("(p t) d -> p (t d)", p=P)  # [128, T*64]
    ohv = oh_h[:, :].rearrange("(p t) d -> p (t d)", p=P)  # [128, T*13]

    # MM sequence: chunks paired within each tile (one 26-col lhsT /
    # 128-col rhs matmul covers 256 points; cross-quadrant garbage lands
    # in PSUM regions the host ignores). Count MMs to place start/stop.
    nmm = sum(tt // 2 + (tt & 1) for _, tt in plan)
    first = [min(m for m in range(nmm) if m % NGRP == g) for g in range(NGRP)]
    last = [max(m for m in range(nmm) if m % NGRP == g) for g in range(NGRP)]

    with tile.TileContext(nc) as tc:
        with (
            tc.tile_pool(name="fin", bufs=1) as fp,
            tc.tile_pool(name="oh", bufs=1) as ohp,
            tc.tile_pool(name="io", bufs=6) as iop,
            tc.tile_pool(name="sq", bufs=4) as sqp,
            tc.tile_pool(name="acc", bufs=1, space="PSUM") as psa,
        ):
            final = fp.tile([P, 2 * D + K], f32)
            nc.vector.memset(final[:, :], 0.0)

            oh_all = ohp.tile([P, T * C], f8)
            acc = psa.tile([32 * (NGRP - 1) + 2 * C, 2 * D], f32)  # [122,128]

            mi = 0
            for i, (t0, tt) in enumerate(plan):
                io = iop.tile([P, TTMAX * D], f8, tag="io")
                nc.sync.dma_start(
                    out=io[:, : tt * D],
                    in_=featv[:, t0 * D : (t0 + tt) * D],
                )
                if i == 0:
                    # one-hots as the 2nd transfer on the same FIFO queue:
                    # land right after the small first feat tile, so the
                    # serial matmul chain starts early
                    nc.sync.dma_start(out=oh_all[:, :], in_=ohv[:, :])
                ks = max(2, int(round(tt * XS)) & ~1)
                sq = sqp.tile([P, TTMAX * D], f8, tag="sq")
                nc.scalar.activation(
                    out=sq[:, : ks * D], in_=io[:, : ks * D],
                    func=mybir.ActivationFunctionType.Square,
                    accum_out=final[:, 2 * D + 2 * i : 2 * D + 2 * i + 1],
                )
                nc.vector.affine_mul_reduce(
                    out=sq[:, ks * D : tt * D],
                    accum_out=final[:, 2 * D + 2 * i + 1 : 2 * D + 2 * i + 2],
                    in0=io[:, ks * D : tt * D],
                    in1=io[:, ks * D : tt * D],
                    scale=1.0,
                    bias=0.0,
                )
                tl = 0
                while tl < tt:
                    g = mi % NGRP
                    w = 2 if tl + 1 < tt else 1
                    nc.tensor.matmul(
                        acc[32 * g : 32 * g + w * C, : w * D],
                        lhsT=oh_all[
                            :, (t0 + tl) * C : (t0 + tl + w) * C
                        ],
                        rhs=io[:, tl * D : (tl + w) * D],
                        start=(mi == first[g]),
                        stop=(mi == last[g]),
                        tile_position=(0, 32 * g),
                    )
                    mi += 1
                    tl += w

            nc.vector.tensor_copy(
                final[0 : 32 * (NGRP - 1) + 2 * C, 0 : 2 * D], acc[:, :]
            )
            nc.sync.dma_start(out=out_h[:, :], in_=final[:, :])
    nc.finalize()
    return nc


# set by test.py to capture profile info
TRACE = False
LAST = {}


def _ensure_ntff_hook():
    """The agent image's antenv lacks axon_hooks; synthesize it so
    run_bass_kernel_spmd(trace=True) can profile. Best-effort."""
    import sys
    import types

    try:
        from antenv.axon_hooks import get_axon_ntff_profile_hook  # noqa: F401
        return
    except ImportError:
        pass
    try:
        from trn_agent_boot.trn_boot import _ntff_profile_via_ctypes

        hook = _ntff_profile_via_ctypes("/opt/axon/libaxon_pjrt.so")
        mod = types.ModuleType("antenv.axon_hooks")
        mod._hook = hook
        mod.get_axon_ntff_profile_hook = lambda: mod._hook
        mod.set_axon_ntff_profile_hook = lambda h: setattr(mod, "_hook", h)
        sys.modules["antenv.axon_hooks"] = mod
        import antenv

        antenv.axon_hooks = mod
    except Exception as e:  # degrade: no profile, run still works
        print(f"ntff hook injection failed: {e}")


def kernel(pred=None, target=None, feat=None, batch=None, centers=None):
    target = np.asarray(target)
    feat = np.asarray(feat, dtype=np.float32)
    batch = np.asarray(batch)
    centers = np.asarray(centers, dtype=np.float64)

    # shard at batch boundaries: core b <- batch b (batch is sorted)
    bounds = np.searchsorted(batch, np.arange(B + 1))
    sizes = np.diff(bounds)
    T = int(max((int(sizes.max()) + P - 1) // P, 8))
    Npad = P * T

    valid_all = target != IGNORE
    feat8 = np.clip(feat, -240.0, 240.0).astype(F8NP)

    in_maps = []
    for b in range(B):
        lo, hi = int(bounds[b]), int(bounds[b + 1])
        n = hi - lo
        fb8 = np.zeros((Npad, D), dtype=F8NP)
        ohb = np.zeros((Npad, C), dtype=F8NP)
        v = valid_all[lo:hi]
        fb = feat8[lo:hi].copy()
        fb[~v] = 0
        fb8[:n] = fb
        tb = target[lo:hi]
        rows = np.nonzero(v)[0]
        ohb[rows, tb[rows]] = 1.0
        in_maps.append({"feat8": fb8, "oh8": ohb})

    nc = build_nc(T)
    if TRACE:
        _ensure_ntff_hook()
    res = run_bass_kernel_spmd(nc, in_maps, list(range(N_CORES)), trace=TRACE)
    LAST["results"] = res

    # host-side (exact, from int inputs only): per-batch/class counts
    seg = (batch.astype(np.int64) * C + np.where(valid_all, target, 0))[
        valid_all
    ]
    ccnt = np.bincount(seg, minlength=B * C).reshape(B, C).astype(np.float64)
    cnt_b = np.bincount(batch[valid_all], minlength=B).astype(np.float64)

    cen_sq = (centers**2).sum(axis=1)  # (13,)
    total_intra = 0.0
    total_inter = 0.0
    n_present = 0
    for b in range(B):
        o = np.asarray(res.results[b]["out"]).astype(np.float64)
        fsum = np.zeros((C, D))
        for g in range(NGRP):
            blk = o[32 * g : 32 * g + 2 * C, 0 : 2 * D]
            fsum += blk[0:C, 0:D] + blk[C : 2 * C, D : 2 * D]
        S = o[:, 2 * D :].sum()
        if cnt_b[b] <= 0:
            continue
        n_present += 1
        tdot = (centers * fsum).sum()
        utot = (ccnt[b] * cen_sq).sum()
        total_intra += (S - 2.0 * tdot + utot) / max(cnt_b[b], 1.0)

        cm = fsum / np.maximum(ccnt[b], 1.0)[:, None]
        pres = ccnt[b] > 0
        dd2 = ((cm[:, None, :] - cm[None, :, :]) ** 2).sum(-1)
        pm = pres[:, None] & pres[None, :] & ~np.eye(C, dtype=bool)
        dist = np.sqrt(np.where(pm, dd2, 1.0))
        terms = np.where(pm, np.maximum(MARGIN - dist, 0.0), 0.0)
        npairs = pm.sum()
        total_inter += terms.sum() / max(npairs, 1)

    den = max(n_present, 1)
    loss = LOSS_W * (
        INTRA_W * total_intra / den + INTER_W * total_inter / den
    )
    return np.float32(loss)
